# revision 15
# baseline (speedup 1.0000x reference)
"""Trainium2 Bass kernel for nn_PrototypeBarlow (vq_codebook).

Sharding (8 cores):
  - Encode (bf16): shard D_IMG (contraction); per-core partial z^T [P_DIM, B]
    per dataset, AllReduce(add) bf16 per dataset; a's AR overlaps b's encode,
    and both ARs hide behind tensor work. Input DMAs chunked per k-tile.
  - Prototypes (fp8 DoubleRow): shard N_PROTO; augmented matmul on
    zaug = [z; 16; z^2/16; 0-pad] vs prp = [-2 proto^T; p2/16; 16; 0-pad]
    (scales keep every fp8 operand under the 240 max-normal).
  - Per-dataset chain zaug -> pt -> stats -> norm -> transpose -> decode
    pipelines against the other dataset's AllReduce.
  - Barlow: transpose Q (TensorE identity matmul) -> [B, 256] fp8; AllGather
    Qb^T in two 128KB chunks; c-band matmul (fp8 DoubleRow) accumulates per
    chunk; square+sum. diag(c) local row dots in bf16.
  - VAE (fp8 DoubleRow decode, W_dec pre-scaled x16): fused
    (dec/16 - ds) via scalar_tensor_tensor, square on ACT engine, reduce on
    DVE.
  - error_1: free-axis min + local sum. error_2: partition-tree min ->
    [1,B], AllGather + local min tree (cheaper than AllReduce-min).
  - Final: pre-scaled scalar partials [1,8], AllGather -> [8,8], ones-matmul.
"""

import numpy as np
import ml_dtypes

BF16 = ml_dtypes.bfloat16
F8 = ml_dtypes.float8_e4m3

B = 1024
D_IMG = 12288
P_DIM = 512
N_PROTO = 2048
NCORES = 8
DSH = D_IMG // NCORES    # 1536
NSH = N_PROTO // NCORES  # 256
KAUG = 768               # 512 + aug rows, padded to 6*128 (even for DoubleRow)
LAMBD = 0.005
EPS = 1e-5
WDS = 16.0               # host pre-scale on W_dec / aug rows for fp8 range

_PROG_CACHE = {}


def _build_program(stage=99):
    import concourse.bacc as bacc
    import concourse.tile as tile
    from concourse import mybir

    class _StageDone(Exception):
        pass

    nc = bacc.Bacc("TRN2", target_bir_lowering=False, num_devices=NCORES)

    try:
        _run_build(nc, tile, mybir, stage, _StageDone)
    except _StageDone:
        pass
    nc.finalize()
    return nc


def _run_build(nc, tile, mybir, stage, _StageDone):
    from contextlib import ExitStack
    from concourse.masks import make_identity

    dt = mybir.dt
    f32 = dt.float32
    bft = dt.bfloat16
    fp8 = dt.float8e4
    AO = mybir.AluOpType
    DR = mybir.MatmulPerfMode.DoubleRow
    P = 128
    RG = [list(range(NCORES))]
    dsa = nc.dram_tensor("dsa", [DSH, B], bft, kind="ExternalInput")
    dsb = nc.dram_tensor("dsb", [DSH, B], bft, kind="ExternalInput")
    wenc = nc.dram_tensor("wenc", [DSH, P_DIM], bft, kind="ExternalInput")
    wdec = nc.dram_tensor("wdec", [P_DIM, DSH], fp8, kind="ExternalInput")
    prp = nc.dram_tensor("prp", [KAUG, NSH], fp8, kind="ExternalInput")
    out = nc.dram_tensor("out", [1, 1], f32, kind="ExternalOutput")

    with tile.TileContext(nc) as tc, ExitStack() as ctx:
      try:
            dram = ctx.enter_context(tc.tile_pool(name="dram", bufs=1, space="DRAM"))
            bZ = {}
            bZR = {}
            for s in "ab":
                bZ[s] = dram.tile([P_DIM, B], bft, name=f"bZ{s}", tag=f"bZ{s}")
                bZR[s] = dram.tile(
                    [P_DIM, B], bft, addr_space="Shared", name=f"bZR{s}", tag=f"bZR{s}"
                )
            # Qb^T gathered in two batch-half chunks
            bQt = {}
            bQtR = {}
            for h in range(2):
                bQt[h] = dram.tile([B // 2, NSH], fp8, name=f"bQt{h}", tag=f"bQt{h}")
                bQtR[h] = dram.tile(
                    [B * NCORES // 2, NSH], fp8, addr_space="Shared",
                    name=f"bQtR{h}", tag=f"bQtR{h}",
                )
            bMin = dram.tile([1, B], f32, name="bMin", tag="bMin")
            bMinR = dram.tile(
                [NCORES, B], f32, addr_space="Shared", name="bMinR", tag="bMinR"
            )
            bPack = dram.tile([1, 8], f32, name="bPack", tag="bPack")
            bPackR = dram.tile(
                [NCORES, 8], f32, addr_space="Shared", name="bPackR", tag="bPackR"
            )

            const = ctx.enter_context(tc.tile_pool(name="const", bufs=1))
            wenc_sb = const.tile([P, 12, P_DIM], bft, name="wenc_sb", tag="wbig")
            wenc_r = wenc[:].rearrange("(ko ki) n -> ki ko n", ki=P)
            for k in range(12):
                nc.sync.dma_start(wenc_sb[:, k, :], wenc_r[:, k, :])
            prp_sb = const.tile([P, 6, NSH], fp8, name="prp_sb", tag="prp_sb")
            nc.sync.dma_start(prp_sb[:], prp[:].rearrange("(ko ki) n -> ki ko n", ki=P))
            ones_col = const.tile([P, 1], bft, name="ones_col", tag="ones_col")
            nc.vector.memset(ones_col[:], 1.0)
            ones_f32 = const.tile([P, 1], f32, name="ones_f32", tag="ones_f32")
            nc.vector.memset(ones_f32[:], 1.0)
            ident = const.tile([P, P], bft, name="ident", tag="ident")
            make_identity(nc, ident[:])

            dsp = ctx.enter_context(tc.tile_pool(name="dsp", bufs=1))
            ds_sb = {}
            for s, t in (("a", dsa), ("b", dsb)):
                ds_sb[s] = dsp.tile([P, 12, B], bft, name=f"ds{s}_sb", tag=f"ds{s}_sb")
                t_r = t[:].rearrange("(ko ki) b -> ki ko b", ki=P)
                for k in range(12):
                    nc.sync.dma_start(ds_sb[s][:, k, :], t_r[:, k, :])

            psum = ctx.enter_context(tc.tile_pool(name="psum", bufs=6, space="PSUM"))
            psa2 = ctx.enter_context(tc.tile_pool(name="psa2", bufs=1, space="PSUM"))
            zp = ctx.enter_context(tc.tile_pool(name="zp", bufs=1))
            protp = ctx.enter_context(tc.tile_pool(name="protp", bufs=1))
            scr = ctx.enter_context(tc.tile_pool(name="scr", bufs=3))
            small = ctx.enter_context(tc.tile_pool(name="small", bufs=1))
            evp = ctx.enter_context(tc.tile_pool(name="evp", bufs=4))

            # per-partition partial sums gathered as columns; reduced once at the end
            sums = small.tile([P, 8], f32, name="sums", tag="sums")
            nc.vector.memset(sums[:], 0.0)
            vacc = small.tile([P, 48], f32, name="vacc", tag="vacc")

            # ---------------- encode:  zT_part = wenc^T @ dsT (bf16) ----------
            for s in "ab":
                src = ds_sb[s]
                bZt = bZ[s][:].rearrange("(ko ki) b -> ki ko b", ki=P)
                for mg in range(2):
                    pts = {}
                    for mi in range(2):
                        for n in range(2):
                            pts[(mi, n)] = psum.tile(
                                [P, 512], f32, tag="mm", name=f"enc_{s}_{mg}_{mi}_{n}"
                            )
                    for k in range(12):
                        for mi in range(2):
                            m = mg * 2 + mi
                            for n in range(2):
                                nc.tensor.matmul(
                                    pts[(mi, n)][:],
                                    wenc_sb[:, k, m * P : (m + 1) * P],
                                    src[:, k, n * 512 : (n + 1) * 512],
                                    start=(k == 0),
                                    stop=(k == 11),
                                )
                    for mi in range(2):
                        m = mg * 2 + mi
                        for n in range(2):
                            ev = evp.tile([P, 512], bft, tag="ev", name=f"ev_{s}_{m}_{n}")
                            nc.any.tensor_copy(out=ev[:], in_=pts[(mi, n)][:])
                            nc.sync.dma_start(bZt[:, m, n * 512 : (n + 1) * 512], ev[:])
                nc.gpsimd.collective_compute(
                    "AllReduce",
                    AO.add,
                    replica_groups=RG,
                    ins=[bZ[s][:]],
                    outs=[bZR[s][:]],
                )

            # wdec reuses wenc's SBUF slot once the encode matmuls are done
            wdec_sb = const.tile([P, 4, DSH], fp8, name="wdec_sb", tag="wbig")
            nc.sync.dma_start(wdec_sb[:], wdec[:].rearrange("(ko ki) n -> ki ko n", ki=P))

            def _dbg_out(ap):
                dbg = small.tile([1, 1], f32, name="dbg", tag="dbg")
                nc.vector.tensor_copy(out=dbg[:], in_=ap)
                nc.sync.dma_start(out[:], dbg[:])

            # ---- per-dataset: zaug -> pt -> stats -> norm -> qT -> decode ----
            zaug = {}
            pt = {}
            q = {}
            qT = {}
            for si, s in enumerate("ab"):
                # zaug (fp8) = [z; 16; a2/16; pad] via bf16 staging
                zst = scr.tile([P, 4, B], bft, tag="zst", name=f"zst_{s}", bufs=2)
                nc.sync.dma_start(
                    zst[:], bZR[s][:].rearrange("(ko ki) b -> ki ko b", ki=P)
                )
                za = zp.tile([P, 6, B], fp8, name=f"zaug_{s}", tag=f"zaug_{s}")
                zaug[s] = za
                nc.vector.tensor_copy(out=za[:, 0:4, :], in_=zst[:])
                nc.vector.memset(za[:, 4:6, :], 0.0)
                nc.vector.memset(za[0:1, 4, :], WDS)
                zsq = scr.tile([P, 4, B], bft, tag="zsq", name=f"zsq_{s}", bufs=2)
                nc.vector.tensor_tensor(
                    out=zsq[:], in0=zst[:], in1=zst[:], op=AO.mult
                )
                pa2 = psa2.tile([1, 2, 512], f32, tag="a2", name=f"a2_{s}")
                for k in range(4):
                    for n in range(2):
                        nc.tensor.matmul(
                            pa2[:, n, :],
                            ones_col[:],
                            zsq[:, k, n * 512 : (n + 1) * 512],
                            start=(k == 0),
                            stop=(k == 3),
                        )
                # a2/16 lands on partition 32 of aug chunk 4 (pairs prp row 544=16)
                nc.vector.tensor_scalar(
                    out=za[32:33, 4, :],
                    in0=pa2[0:1, :, :],
                    scalar1=1.0 / WDS,
                    scalar2=None,
                    op0=AO.mult,
                )

                # protT = prp^T @ zaug   [256, B] f32   (fp8 DoubleRow, K=768)
                ptile = protp.tile([P, 2, B], f32, name=f"pt_{s}", tag=f"pt_{s}")
                pt[s] = ptile
                for m in range(2):
                    pps = {}
                    for n in range(2):
                        pps[n] = psum.tile([P, 512], f32, tag="mm", name=f"pr_{s}_{m}_{n}")
                    for kg in range(3):
                        for n in range(2):
                            nc.tensor.matmul(
                                pps[n][:],
                                prp_sb[:, 2 * kg : 2 * kg + 2, m * P : (m + 1) * P],
                                zaug[s][:, 2 * kg : 2 * kg + 2, n * 512 : (n + 1) * 512],
                                start=(kg == 0),
                                stop=(kg == 2),
                                perf_mode=DR,
                            )
                    for n in range(2):
                        nc.any.tensor_copy(
                            out=ptile[:, m, n * 512 : (n + 1) * 512], in_=pps[n][:]
                        )

                # barlow stats + normalize
                qt = protp.tile([P, 2, B], bft, name=f"q_{s}", tag=f"q_{s}")
                q[s] = qt
                for m in range(2):
                    st6 = small.tile(
                        [P, 2, 6], f32, tag="st6", name=f"st6_{s}_{m}", bufs=2
                    )
                    for c in range(2):
                        nc.vector.bn_stats(
                            out=st6[:, c, :], in_=pt[s][:, m, c * 512 : (c + 1) * 512]
                        )
                    mv = small.tile([P, 2], f32, tag="mv", name=f"mv_{s}_{m}", bufs=2)
                    nc.vector.bn_aggr(out=mv[:], in_=st6[:])
                    sd = small.tile([P, 1], f32, tag="sd", name=f"sd_{s}_{m}", bufs=2)
                    nc.scalar.sqrt(out=sd[:], in_=mv[:, 1:2])
                    sde = small.tile([P, 1], f32, tag="sde", name=f"sde_{s}_{m}", bufs=2)
                    nc.vector.tensor_scalar(
                        out=sde[:], in0=sd[:], scalar1=EPS, scalar2=None, op0=AO.add
                    )
                    rstd = small.tile([P, 1], f32, tag="rstd", name=f"rstd_{s}_{m}", bufs=2)
                    nc.vector.reciprocal(out=rstd[:], in_=sde[:])
                    nc.vector.tensor_scalar(
                        out=qt[:, m, :],
                        in0=pt[s][:, m, :],
                        scalar1=mv[:, 0:1],
                        scalar2=rstd[:],
                        op0=AO.subtract,
                        op1=AO.mult,
                    )

                # transpose q -> qT [B-part, 256] fp8
                qT[s] = protp.tile([P, 8, NSH], fp8, name=f"qT_{s}", tag=f"qT_{s}")
                for m in range(2):
                    for g in range(2):
                        ptr = psum.tile(
                            [P, 4, P], bft, tag="mm", name=f"tp_{s}_{m}_{g}"
                        )
                        for kk in range(4):
                            kb = g * 4 + kk
                            nc.tensor.transpose(
                                ptr[:, kk, :],
                                q[s][:, m, kb * P : (kb + 1) * P],
                                ident[:],
                            )
                        nc.any.tensor_copy(
                            out=qT[s][:, g * 4 : (g + 1) * 4, m * P : (m + 1) * P],
                            in_=ptr[:],
                        )

                if s == "b":
                    # ship Qb^T for the AllGather in two batch-half chunks
                    for h in range(2):
                        nc.sync.dma_start(
                            bQt[h][:].rearrange("(ko ki) n -> ki ko n", ki=P),
                            qT["b"][:, 4 * h : 4 * h + 4, :],
                        )
                        nc.gpsimd.collective_compute(
                            "AllGather",
                            AO.bypass,
                            replica_groups=RG,
                            ins=[bQt[h][:]],
                            outs=[bQtR[h][:]],
                        )

                # VAE decode (fp8 DoubleRow, W_dec scaled x16) + fused evac
                for m in range(12):
                    pps = {}
                    for n in range(2):
                        pps[n] = psum.tile([P, 512], f32, tag="mm", name=f"d_{s}_{m}_{n}")
                    for kg in range(2):
                        for n in range(2):
                            nc.tensor.matmul(
                                pps[n][:],
                                wdec_sb[:, 2 * kg : 2 * kg + 2, m * P : (m + 1) * P],
                                zaug[s][:, 2 * kg : 2 * kg + 2, n * 512 : (n + 1) * 512],
                                start=(kg == 0),
                                stop=(kg == 1),
                                perf_mode=DR,
                            )
                    for n in range(2):
                        df = scr.tile([P, 512], bft, tag="df", name=f"df_{s}_{m}_{n}")
                        nc.vector.scalar_tensor_tensor(
                            out=df[:],
                            in0=pps[n][:],
                            scalar=1.0 / WDS,
                            in1=ds_sb[s][:, m, n * 512 : (n + 1) * 512],
                            op0=AO.mult,
                            op1=AO.subtract,
                        )
                        dfs = scr.tile([P, 512], f32, tag="dfs", name=f"dfs_{s}_{m}_{n}")
                        col = si * 24 + m * 2 + n
                        nc.scalar.square(out=dfs[:], in_=df[:])
                        nc.vector.tensor_reduce(
                            out=vacc[:, col : col + 1],
                            in_=dfs[:],
                            axis=mybir.AxisListType.X,
                            op=AO.add,
                        )

            nc.vector.tensor_reduce(
                out=sums[:, 3:4], in_=vacc[:], axis=mybir.AxisListType.X, op=AO.add
            )

            if stage <= 1:
                _dbg_out(zaug["b"][0:1, 0, 0:1])
                raise _StageDone()
            if stage <= 2:
                _dbg_out(pt["b"][0:1, 0, 0:1])
                raise _StageDone()

            # ---------------- mins on s = prot_a + prot_b ---------------------
            sT = scr.tile([P, 2, B], f32, tag="zst", name="sT", bufs=2)
            minb = small.tile([P, 2], f32, name="minb", tag="minb")
            for m in range(2):
                nc.vector.tensor_tensor(
                    out=sT[:, m, :],
                    in0=pt["a"][:, m, :],
                    in1=pt["b"][:, m, :],
                    op=AO.add,
                )
                nc.vector.tensor_reduce(
                    out=minb[:, m : m + 1],
                    in_=sT[:, m, :],
                    axis=mybir.AxisListType.X,
                    op=AO.min,
                )
            # error_1 partial: sum over local prototypes of min over batch
            nc.vector.tensor_reduce(
                out=sums[:, 0:1], in_=minb[:], axis=mybir.AxisListType.X, op=AO.add
            )
            if stage == 30:
                _dbg_out(minb[0:1, 0:1])
                raise _StageDone()
            # error_2: min over local protos across partitions -> [1, B]:
            # fold 128->32, then 32x32 stream-transpose + free-axis min
            m128 = scr.tile([P, B], f32, tag="m128", name="m128")
            nc.vector.tensor_tensor(
                out=m128[:], in0=sT[:, 0, :], in1=sT[:, 1, :], op=AO.min
            )
            h64 = scr.tile([64, B], f32, tag="m128", name="h64")
            nc.vector.tensor_copy(out=h64[:], in_=m128[64:128, :])
            m64 = scr.tile([64, B], f32, tag="m128", name="m64")
            nc.vector.tensor_tensor(
                out=m64[:], in0=m128[0:64, :], in1=h64[:], op=AO.min
            )
            h32 = scr.tile([32, B], f32, tag="m128", name="h32")
            nc.vector.tensor_copy(out=h32[:], in_=m64[32:64, :])
            m32 = scr.tile([32, B], f32, tag="m128", name="m32")
            nc.vector.tensor_tensor(
                out=m32[:], in0=m64[0:32, :], in1=h32[:], op=AO.min
            )
            m32t = scr.tile([32, B], f32, tag="m128", name="m32t")
            nc.vector.transpose(out=m32t[:], in_=m32[:])
            # m32t[q, j*32 + r] = m32[r, j*32 + q]; reduce r -> min over partitions
            res32 = small.tile([32, 32], f32, name="res32", tag="res32")
            nc.vector.tensor_reduce(
                out=res32[:],
                in_=m32t[:].rearrange("p (j r) -> p j r", r=32),
                axis=mybir.AxisListType.X,
                op=AO.min,
            )
            if stage == 31:
                _dbg_out(res32[0:1, 0:1])
                raise _StageDone()
            # column c = j*32 + q of the original lives at res32[q, j]
            nc.sync.dma_start(
                bMin[:].rearrange("o (j q) -> (o q) j", q=32), res32[:]
            )
            nc.gpsimd.collective_compute(
                "AllGather", AO.bypass, replica_groups=RG,
                ins=[bMin[:]], outs=[bMinR[:]],
            )

            if stage <= 3:
                _dbg_out(res32[0:1, 0:1])
                raise _StageDone()

            # diag(c) local: row dots of Qa^T o Qb^T
            cd = small.tile([P, 2], f32, name="cd", tag="cd")
            for m in range(2):
                cscr = scr.tile([P, B], f32, tag="m128", name=f"cscr_{m}")
                nc.vector.tensor_tensor(
                    out=cscr[:], in0=q["a"][:, m, :], in1=q["b"][:, m, :], op=AO.mult
                )
                nc.vector.tensor_reduce(
                    out=cd[:, m : m + 1],
                    in_=cscr[:],
                    axis=mybir.AxisListType.X,
                    op=AO.add,
                )
            cdn = small.tile([P, 2], f32, name="cdn", tag="cdn")
            nc.vector.tensor_scalar(
                out=cdn[:], in0=cd[:], scalar1=1.0 / B, scalar2=None, op0=AO.mult
            )
            cm1 = small.tile([P, 2], f32, name="cm1", tag="cm1")
            nc.vector.tensor_scalar(
                out=cm1[:], in0=cdn[:], scalar1=1.0, scalar2=None, op0=AO.subtract
            )
            od2 = small.tile([P, 2], f32, name="od2", tag="od2")
            nc.vector.tensor_tensor(out=od2[:], in0=cm1[:], in1=cm1[:], op=AO.mult)
            dsq2 = small.tile([P, 2], f32, name="dsq2", tag="dsq2")
            nc.vector.tensor_tensor(out=dsq2[:], in0=cdn[:], in1=cdn[:], op=AO.mult)
            nc.vector.tensor_reduce(
                out=sums[:, 1:2], in_=od2[:], axis=mybir.AxisListType.X, op=AO.add
            )
            nc.vector.tensor_reduce(
                out=sums[:, 2:3], in_=dsq2[:], axis=mybir.AxisListType.X, op=AO.add
            )

            if stage <= 4:
                _dbg_out(q["b"][0:1, 0, 0:1])
                raise _StageDone()
            if stage <= 5:
                _dbg_out(sums[0:1, 0:1])
                raise _StageDone()

            # ------------- c band: Qa_loc @ QbT_full (fp8 DoubleRow) ----------
            # accumulate over the two gathered batch-half chunks
            qbF = protp.tile([P, 8, N_PROTO], fp8, name="qbF", tag="qbF")
            for h in range(2):
                for r in range(NCORES):
                    nc.sync.dma_start(
                        qbF[:, 4 * h : 4 * h + 4, r * NSH : (r + 1) * NSH],
                        bQtR[h][r * (B // 2) : (r + 1) * (B // 2), :].rearrange(
                            "(ko ki) n -> ki ko n", ki=P
                        ),
                    )
            cacc = small.tile([P, 8], f32, name="cacc", tag="cacc")
            for m in range(2):
                for nh in range(4):
                    pcs = psum.tile([P, 512], f32, tag="mm", name=f"c_{m}_{nh}")
                    for h in range(2):
                        for kg in range(2):
                            kk = 4 * h + 2 * kg
                            nc.tensor.matmul(
                                pcs[:],
                                qT["a"][:, kk : kk + 2, m * P : (m + 1) * P],
                                qbF[:, kk : kk + 2, nh * 512 : (nh + 1) * 512],
                                start=(h == 0 and kg == 0),
                                stop=(h == 1 and kg == 1),
                                perf_mode=DR,
                            )
                    csq = scr.tile([P, 512], f32, tag="dfs", name=f"csq_{m}_{nh}")
                    nc.scalar.square(out=csq[:], in_=pcs[:])
                    nc.vector.tensor_reduce(
                        out=cacc[:, m * 4 + nh : m * 4 + nh + 1],
                        in_=csq[:],
                        axis=mybir.AxisListType.X,
                        op=AO.add,
                    )
            nc.vector.tensor_reduce(
                out=sums[:, 4:5], in_=cacc[:], axis=mybir.AxisListType.X, op=AO.add
            )

            if stage <= 6:
                _dbg_out(sums[0:1, 0:1])
                raise _StageDone()

            # ------------- error_2: min over gathered per-core mins -----------
            # bMinR[r, j*32+q] -> gm32[q, r, j]; reduce r (cores), then sum
            gm32 = small.tile([32, NCORES, 32], f32, name="gm32", tag="gm32")
            nc.sync.dma_start(
                gm32[:], bMinR[:].rearrange("r (j q) -> q r j", q=32)
            )
            rsb = small.tile([32, 32], f32, name="rsb", tag="rsb")
            nc.vector.tensor_reduce(
                out=rsb[:],
                in_=gm32[:].rearrange("q r j -> q j r"),
                axis=mybir.AxisListType.X,
                op=AO.min,
            )
            nc.vector.tensor_reduce(
                out=sums[0:32, 5:6], in_=rsb[:], axis=mybir.AxisListType.X, op=AO.add
            )

            # ------------- pack + final AllGather + local reduce --------------
            fin = psa2.tile([1, 8], f32, tag="a2", name="fin")
            nc.tensor.matmul(
                fin[:],
                ones_f32[:],
                sums[:],
                start=True,
                stop=True,
            )
            pack = small.tile([1, 8], f32, name="pack", tag="pack")
            nc.vector.memset(pack[:], 0.0)
            for col, (part, coef) in enumerate(
                [
                    (fin[0:1, 0:1], 1.0 / N_PROTO),         # error_1
                    (fin[0:1, 5:6], 1.0 / (B * NCORES)),    # error_2 (replicated)
                    (fin[0:1, 3:4], 1.0 / B),               # vae
                    (fin[0:1, 1:2], 1.0),                   # on_diag
                    (fin[0:1, 4:5], LAMBD / (B * B)),       # lambd * sum c^2
                    (fin[0:1, 2:3], -LAMBD),                # -lambd * sum diag^2
                ]
            ):
                nc.vector.tensor_scalar(
                    out=pack[:, col : col + 1],
                    in0=part,
                    scalar1=coef,
                    scalar2=None,
                    op0=AO.mult,
                )
            nc.sync.dma_start(bPack[:], pack[:])
            nc.gpsimd.collective_compute(
                "AllGather",
                AO.bypass,
                replica_groups=RG,
                ins=[bPack[:]],
                outs=[bPackR[:]],
            )
            pr = small.tile([8, 8], f32, name="pr", tag="pr")
            nc.sync.dma_start(pr[:], bPackR[:])
            prr = psa2.tile([1, 8], f32, tag="a2", name="prr")
            nc.tensor.matmul(
                prr[:],
                ones_f32[0:8, :],
                pr[:],
                start=True,
                stop=True,
            )
            res = small.tile([1, 1], f32, name="res", tag="res")
            nc.vector.tensor_reduce(
                out=res[:], in_=prr[:], axis=mybir.AxisListType.X, op=AO.add
            )
            nc.sync.dma_start(out[:], res[:])

      except _StageDone:
          pass
    return


def _get_program(stage=99):
    key = ("nc", stage)
    if key not in _PROG_CACHE:
        _PROG_CACHE[key] = _build_program(stage)
    return _PROG_CACHE[key]


def _make_in_maps(ds_one, ds_two, W_enc, W_dec, prototypes):
    p2 = (prototypes * prototypes).sum(axis=1)
    in_maps = []
    for c in range(NCORES):
        dsl = slice(c * DSH, (c + 1) * DSH)
        nsl = slice(c * NSH, (c + 1) * NSH)
        prp = np.zeros((KAUG, NSH), np.float32)
        prp[0:P_DIM, :] = -2.0 * prototypes[nsl, :].T
        prp[P_DIM, :] = p2[nsl] / WDS        # pairs zaug's 16-row
        prp[P_DIM + 32, :] = WDS             # pairs zaug's a2/16 row
        in_maps.append(
            {
                "dsa": np.ascontiguousarray(ds_one[:, dsl].T).astype(BF16),
                "dsb": np.ascontiguousarray(ds_two[:, dsl].T).astype(BF16),
                "wenc": np.ascontiguousarray(W_enc[dsl, :]).astype(BF16),
                "wdec": np.ascontiguousarray(W_dec[:, dsl] * WDS).astype(F8),
                "prp": prp.astype(F8),
            }
        )
    return in_maps


def kernel(ds_one, ds_two, W_enc, W_dec, prototypes, _trace=False, _tmpdir=None):
    import os
    from concourse import bass_utils

    ds_one = np.asarray(ds_one, np.float32)
    ds_two = np.asarray(ds_two, np.float32)
    W_enc = np.asarray(W_enc, np.float32)
    W_dec = np.asarray(W_dec, np.float32)
    prototypes = np.asarray(prototypes, np.float32)

    nc = _get_program(int(os.environ.get("KSTAGE", "99")))
    in_maps = _make_in_maps(ds_one, ds_two, W_enc, W_dec, prototypes)
    res = bass_utils.run_bass_kernel_spmd(
        nc,
        in_maps,
        core_ids=list(range(NCORES)),
        trace=_trace,
        tmpdir=_tmpdir,
    )
    val = np.asarray(res.results[0]["out"], np.float32)
    if _trace:
        kernel.last_exec_time_ns = res.exec_time_ns
        kernel.last_profile = res.profile_json
    return val.reshape(())


# revision 17
# speedup vs baseline: 1.0034x; 1.0034x over previous
"""Trainium2 Bass kernel for nn_PrototypeBarlow (vq_codebook).

Sharding (8 cores):
  - Encode (bf16): shard D_IMG (contraction); per-core partial z^T [P_DIM, B]
    per dataset, AllReduce(add) bf16 per dataset; a's AR overlaps b's encode,
    and both ARs hide behind tensor work. Input DMAs chunked per k-tile.
  - Prototypes (fp8 DoubleRow): shard N_PROTO; augmented matmul on
    zaug = [z; 16; z^2/16; 0-pad] vs prp = [-2 proto^T; p2/16; 16; 0-pad]
    (scales keep every fp8 operand under the 240 max-normal).
  - Per-dataset chain zaug -> pt -> stats -> norm -> transpose -> decode
    pipelines against the other dataset's AllReduce.
  - Barlow: transpose Q (TensorE identity matmul) -> [B, 256] fp8; AllGather
    Qb^T in two 128KB chunks; c-band matmul (fp8 DoubleRow) accumulates per
    chunk; square+sum. diag(c) local row dots in bf16.
  - VAE (fp8 DoubleRow decode, W_dec pre-scaled x16): fused
    (dec/16 - ds) via scalar_tensor_tensor, square on ACT engine, reduce on
    DVE.
  - error_1: free-axis min + local sum. error_2: partition-tree min ->
    [1,B], AllGather + local min tree (cheaper than AllReduce-min).
  - Final: pre-scaled scalar partials [1,8], AllGather -> [8,8], ones-matmul.
"""

import numpy as np
import ml_dtypes

BF16 = ml_dtypes.bfloat16
F8 = ml_dtypes.float8_e4m3

B = 1024
D_IMG = 12288
P_DIM = 512
N_PROTO = 2048
NCORES = 8
DSH = D_IMG // NCORES    # 1536
NSH = N_PROTO // NCORES  # 256
KAUG = 768               # 512 + aug rows, padded to 6*128 (even for DoubleRow)
LAMBD = 0.005
EPS = 1e-5
WDS = 16.0               # host pre-scale on W_dec / aug rows for fp8 range

_PROG_CACHE = {}


def _build_program(stage=99):
    import concourse.bacc as bacc
    import concourse.tile as tile
    from concourse import mybir

    class _StageDone(Exception):
        pass

    nc = bacc.Bacc("TRN2", target_bir_lowering=False, num_devices=NCORES)

    try:
        _run_build(nc, tile, mybir, stage, _StageDone)
    except _StageDone:
        pass
    nc.finalize()
    return nc


def _run_build(nc, tile, mybir, stage, _StageDone):
    from contextlib import ExitStack
    from concourse.masks import make_identity

    dt = mybir.dt
    f32 = dt.float32
    bft = dt.bfloat16
    fp8 = dt.float8e4
    AO = mybir.AluOpType
    DR = mybir.MatmulPerfMode.DoubleRow
    P = 128
    RG = [list(range(NCORES))]
    dsa = nc.dram_tensor("dsa", [DSH, B], bft, kind="ExternalInput")
    dsb = nc.dram_tensor("dsb", [DSH, B], bft, kind="ExternalInput")
    wenc = nc.dram_tensor("wenc", [DSH, P_DIM], bft, kind="ExternalInput")
    wdec = nc.dram_tensor("wdec", [P_DIM, DSH], fp8, kind="ExternalInput")
    prp = nc.dram_tensor("prp", [KAUG, NSH], fp8, kind="ExternalInput")
    out = nc.dram_tensor("out", [1, 1], f32, kind="ExternalOutput")

    with tile.TileContext(nc) as tc, ExitStack() as ctx:
      try:
            dram = ctx.enter_context(tc.tile_pool(name="dram", bufs=1, space="DRAM"))
            bZ = {}
            bZR = {}
            for s in "ab":
                bZ[s] = dram.tile([P_DIM, B], bft, name=f"bZ{s}", tag=f"bZ{s}")
                bZR[s] = dram.tile(
                    [P_DIM, B], bft, addr_space="Shared", name=f"bZR{s}", tag=f"bZR{s}"
                )
            # Qb^T gathered in two batch-half chunks
            bQt = {}
            bQtR = {}
            for h in range(2):
                bQt[h] = dram.tile([B // 2, NSH], fp8, name=f"bQt{h}", tag=f"bQt{h}")
                bQtR[h] = dram.tile(
                    [B * NCORES // 2, NSH], fp8, addr_space="Shared",
                    name=f"bQtR{h}", tag=f"bQtR{h}",
                )
            bMin = dram.tile([1, B], f32, name="bMin", tag="bMin")
            bMinR = dram.tile(
                [NCORES, B], f32, addr_space="Shared", name="bMinR", tag="bMinR"
            )
            bPack = dram.tile([1, 8], f32, name="bPack", tag="bPack")
            bPackR = dram.tile(
                [NCORES, 8], f32, addr_space="Shared", name="bPackR", tag="bPackR"
            )

            const = ctx.enter_context(tc.tile_pool(name="const", bufs=1))
            wenc_sb = const.tile([P, 12, P_DIM], bft, name="wenc_sb", tag="wbig")
            wenc_r = wenc[:].rearrange("(ko ki) n -> ki ko n", ki=P)
            for k in range(12):
                nc.sync.dma_start(wenc_sb[:, k, :], wenc_r[:, k, :])
            prp_sb = const.tile([P, 6, NSH], fp8, name="prp_sb", tag="prp_sb")
            nc.sync.dma_start(prp_sb[:], prp[:].rearrange("(ko ki) n -> ki ko n", ki=P))
            ones_col = const.tile([P, 1], bft, name="ones_col", tag="ones_col")
            nc.vector.memset(ones_col[:], 1.0)
            ones_f32 = const.tile([P, 1], f32, name="ones_f32", tag="ones_f32")
            nc.vector.memset(ones_f32[:], 1.0)
            ident = const.tile([P, P], bft, name="ident", tag="ident")
            make_identity(nc, ident[:])

            dsp = ctx.enter_context(tc.tile_pool(name="dsp", bufs=1))
            ds_sb = {}
            for s in "ab":
                ds_sb[s] = dsp.tile([P, 12, B], bft, name=f"ds{s}_sb", tag=f"ds{s}_sb")
            dsa_r = dsa[:].rearrange("(ko ki) b -> ki ko b", ki=P)
            for k in range(12):
                nc.sync.dma_start(ds_sb["a"][:, k, :], dsa_r[:, k, :])

            psum = ctx.enter_context(tc.tile_pool(name="psum", bufs=6, space="PSUM"))
            psa2 = ctx.enter_context(tc.tile_pool(name="psa2", bufs=1, space="PSUM"))
            zp = ctx.enter_context(tc.tile_pool(name="zp", bufs=1))
            protp = ctx.enter_context(tc.tile_pool(name="protp", bufs=1))
            scr = ctx.enter_context(tc.tile_pool(name="scr", bufs=3))
            small = ctx.enter_context(tc.tile_pool(name="small", bufs=1))
            evp = ctx.enter_context(tc.tile_pool(name="evp", bufs=4))

            # per-partition partial sums gathered as columns; reduced once at the end
            sums = small.tile([P, 8], f32, name="sums", tag="sums")
            nc.vector.memset(sums[:], 0.0)
            vacc = small.tile([P, 48], f32, name="vacc", tag="vacc")

            # ---------------- encode:  zT_part = wenc^T @ dsT (bf16) ----------
            for s in "ab":
                src = ds_sb[s]
                bZt = bZ[s][:].rearrange("(ko ki) b -> ki ko b", ki=P)
                for mg in range(2):
                    pts = {}
                    for mi in range(2):
                        for n in range(2):
                            pts[(mi, n)] = psum.tile(
                                [P, 512], f32, tag="mm", name=f"enc_{s}_{mg}_{mi}_{n}"
                            )
                    for k in range(12):
                        for mi in range(2):
                            m = mg * 2 + mi
                            for n in range(2):
                                nc.tensor.matmul(
                                    pts[(mi, n)][:],
                                    wenc_sb[:, k, m * P : (m + 1) * P],
                                    src[:, k, n * 512 : (n + 1) * 512],
                                    start=(k == 0),
                                    stop=(k == 11),
                                )
                    for mi in range(2):
                        m = mg * 2 + mi
                        for n in range(2):
                            ev = evp.tile([P, 512], bft, tag="ev", name=f"ev_{s}_{m}_{n}")
                            nc.any.tensor_copy(out=ev[:], in_=pts[(mi, n)][:])
                            nc.sync.dma_start(bZt[:, m, n * 512 : (n + 1) * 512], ev[:])
                nc.gpsimd.collective_compute(
                    "AllReduce",
                    AO.add,
                    replica_groups=RG,
                    ins=[bZ[s][:]],
                    outs=[bZR[s][:]],
                )
                if s == "a":
                    dsb_r = dsb[:].rearrange("(ko ki) b -> ki ko b", ki=P)
                    for k in range(12):
                        nc.sync.dma_start(ds_sb["b"][:, k, :], dsb_r[:, k, :])

            # wdec reuses wenc's SBUF slot once the encode matmuls are done
            wdec_sb = const.tile([P, 4, DSH], fp8, name="wdec_sb", tag="wbig")
            nc.sync.dma_start(wdec_sb[:], wdec[:].rearrange("(ko ki) n -> ki ko n", ki=P))

            def _dbg_out(ap):
                dbg = small.tile([1, 1], f32, name="dbg", tag="dbg")
                nc.vector.tensor_copy(out=dbg[:], in_=ap)
                nc.sync.dma_start(out[:], dbg[:])

            # ---- per-dataset: zaug -> pt -> stats -> norm -> qT -> decode ----
            zaug = {}
            pt = {}
            q = {}
            qT = {}
            for si, s in enumerate("ab"):
                # zaug (fp8) = [z; 16; a2/16; pad] via bf16 staging
                zst = scr.tile([P, 4, B], bft, tag="zst", name=f"zst_{s}", bufs=2)
                nc.sync.dma_start(
                    zst[:], bZR[s][:].rearrange("(ko ki) b -> ki ko b", ki=P)
                )
                za = zp.tile([P, 6, B], fp8, name=f"zaug_{s}", tag=f"zaug_{s}")
                zaug[s] = za
                nc.vector.tensor_copy(out=za[:, 0:4, :], in_=zst[:])
                nc.vector.memset(za[:, 4:6, :], 0.0)
                nc.vector.memset(za[0:1, 4, :], WDS)
                zsq = scr.tile([P, 4, B], bft, tag="zsq", name=f"zsq_{s}", bufs=2)
                nc.vector.tensor_tensor(
                    out=zsq[:], in0=zst[:], in1=zst[:], op=AO.mult
                )
                pa2 = psa2.tile([1, 2, 512], f32, tag="a2", name=f"a2_{s}")
                for k in range(4):
                    for n in range(2):
                        nc.tensor.matmul(
                            pa2[:, n, :],
                            ones_col[:],
                            zsq[:, k, n * 512 : (n + 1) * 512],
                            start=(k == 0),
                            stop=(k == 3),
                        )
                # a2/16 lands on partition 32 of aug chunk 4 (pairs prp row 544=16)
                nc.vector.tensor_scalar(
                    out=za[32:33, 4, :],
                    in0=pa2[0:1, :, :],
                    scalar1=1.0 / WDS,
                    scalar2=None,
                    op0=AO.mult,
                )

                # protT = prp^T @ zaug   [256, B] f32   (fp8 DoubleRow, K=768)
                ptile = protp.tile([P, 2, B], f32, name=f"pt_{s}", tag=f"pt_{s}")
                pt[s] = ptile
                for m in range(2):
                    pps = {}
                    for n in range(2):
                        pps[n] = psum.tile([P, 512], f32, tag="mm", name=f"pr_{s}_{m}_{n}")
                    for kg in range(3):
                        for n in range(2):
                            nc.tensor.matmul(
                                pps[n][:],
                                prp_sb[:, 2 * kg : 2 * kg + 2, m * P : (m + 1) * P],
                                zaug[s][:, 2 * kg : 2 * kg + 2, n * 512 : (n + 1) * 512],
                                start=(kg == 0),
                                stop=(kg == 2),
                                perf_mode=DR,
                            )
                    for n in range(2):
                        nc.any.tensor_copy(
                            out=ptile[:, m, n * 512 : (n + 1) * 512], in_=pps[n][:]
                        )

                # barlow stats + normalize
                qt = protp.tile([P, 2, B], bft, name=f"q_{s}", tag=f"q_{s}")
                q[s] = qt
                for m in range(2):
                    st6 = small.tile(
                        [P, 2, 6], f32, tag="st6", name=f"st6_{s}_{m}", bufs=2
                    )
                    for c in range(2):
                        nc.vector.bn_stats(
                            out=st6[:, c, :], in_=pt[s][:, m, c * 512 : (c + 1) * 512]
                        )
                    mv = small.tile([P, 2], f32, tag="mv", name=f"mv_{s}_{m}", bufs=2)
                    nc.vector.bn_aggr(out=mv[:], in_=st6[:])
                    sd = small.tile([P, 1], f32, tag="sd", name=f"sd_{s}_{m}", bufs=2)
                    nc.scalar.sqrt(out=sd[:], in_=mv[:, 1:2])
                    sde = small.tile([P, 1], f32, tag="sde", name=f"sde_{s}_{m}", bufs=2)
                    nc.vector.tensor_scalar(
                        out=sde[:], in0=sd[:], scalar1=EPS, scalar2=None, op0=AO.add
                    )
                    rstd = small.tile([P, 1], f32, tag="rstd", name=f"rstd_{s}_{m}", bufs=2)
                    nc.vector.reciprocal(out=rstd[:], in_=sde[:])
                    nc.vector.tensor_scalar(
                        out=qt[:, m, :],
                        in0=pt[s][:, m, :],
                        scalar1=mv[:, 0:1],
                        scalar2=rstd[:],
                        op0=AO.subtract,
                        op1=AO.mult,
                    )

                # transpose q -> qT [B-part, 256] fp8
                qT[s] = protp.tile([P, 8, NSH], fp8, name=f"qT_{s}", tag=f"qT_{s}")
                for m in range(2):
                    for g in range(2):
                        ptr = psum.tile(
                            [P, 4, P], bft, tag="mm", name=f"tp_{s}_{m}_{g}"
                        )
                        for kk in range(4):
                            kb = g * 4 + kk
                            nc.tensor.transpose(
                                ptr[:, kk, :],
                                q[s][:, m, kb * P : (kb + 1) * P],
                                ident[:],
                            )
                        nc.any.tensor_copy(
                            out=qT[s][:, g * 4 : (g + 1) * 4, m * P : (m + 1) * P],
                            in_=ptr[:],
                        )

                if s == "b":
                    # ship Qb^T for the AllGather in two batch-half chunks
                    for h in range(2):
                        nc.sync.dma_start(
                            bQt[h][:].rearrange("(ko ki) n -> ki ko n", ki=P),
                            qT["b"][:, 4 * h : 4 * h + 4, :],
                        )
                        nc.gpsimd.collective_compute(
                            "AllGather",
                            AO.bypass,
                            replica_groups=RG,
                            ins=[bQt[h][:]],
                            outs=[bQtR[h][:]],
                        )

            # ---- VAE decode (fp8 DoubleRow) for both datasets; placed after
            # the AllGather trigger so the gather hides behind these matmuls
            for si, s in enumerate("ab"):
                for m in range(12):
                    pps = {}
                    for n in range(2):
                        pps[n] = psum.tile([P, 512], f32, tag="mm", name=f"d_{s}_{m}_{n}")
                    for kg in range(2):
                        for n in range(2):
                            nc.tensor.matmul(
                                pps[n][:],
                                wdec_sb[:, 2 * kg : 2 * kg + 2, m * P : (m + 1) * P],
                                zaug[s][:, 2 * kg : 2 * kg + 2, n * 512 : (n + 1) * 512],
                                start=(kg == 0),
                                stop=(kg == 1),
                                perf_mode=DR,
                            )
                    for n in range(2):
                        df = scr.tile([P, 512], bft, tag="df", name=f"df_{s}_{m}_{n}")
                        nc.vector.scalar_tensor_tensor(
                            out=df[:],
                            in0=pps[n][:],
                            scalar=1.0 / WDS,
                            in1=ds_sb[s][:, m, n * 512 : (n + 1) * 512],
                            op0=AO.mult,
                            op1=AO.subtract,
                        )
                        dfs = scr.tile([P, 512], f32, tag="dfs", name=f"dfs_{s}_{m}_{n}")
                        col = si * 24 + m * 2 + n
                        nc.scalar.square(out=dfs[:], in_=df[:])
                        nc.vector.tensor_reduce(
                            out=vacc[:, col : col + 1],
                            in_=dfs[:],
                            axis=mybir.AxisListType.X,
                            op=AO.add,
                        )

            nc.vector.tensor_reduce(
                out=sums[:, 3:4], in_=vacc[:], axis=mybir.AxisListType.X, op=AO.add
            )

            if stage <= 1:
                _dbg_out(zaug["b"][0:1, 0, 0:1])
                raise _StageDone()
            if stage <= 2:
                _dbg_out(pt["b"][0:1, 0, 0:1])
                raise _StageDone()

            # ---------------- mins on s = prot_a + prot_b ---------------------
            sT = scr.tile([P, 2, B], f32, tag="zst", name="sT", bufs=2)
            minb = small.tile([P, 2], f32, name="minb", tag="minb")
            for m in range(2):
                nc.vector.tensor_tensor(
                    out=sT[:, m, :],
                    in0=pt["a"][:, m, :],
                    in1=pt["b"][:, m, :],
                    op=AO.add,
                )
                nc.vector.tensor_reduce(
                    out=minb[:, m : m + 1],
                    in_=sT[:, m, :],
                    axis=mybir.AxisListType.X,
                    op=AO.min,
                )
            # error_1 partial: sum over local prototypes of min over batch
            nc.vector.tensor_reduce(
                out=sums[:, 0:1], in_=minb[:], axis=mybir.AxisListType.X, op=AO.add
            )
            if stage == 30:
                _dbg_out(minb[0:1, 0:1])
                raise _StageDone()
            # error_2: min over local protos across partitions -> [1, B]:
            # fold 128->32, then 32x32 stream-transpose + free-axis min
            m128 = scr.tile([P, B], f32, tag="m128", name="m128")
            nc.vector.tensor_tensor(
                out=m128[:], in0=sT[:, 0, :], in1=sT[:, 1, :], op=AO.min
            )
            h64 = scr.tile([64, B], f32, tag="m128", name="h64")
            nc.vector.tensor_copy(out=h64[:], in_=m128[64:128, :])
            m64 = scr.tile([64, B], f32, tag="m128", name="m64")
            nc.vector.tensor_tensor(
                out=m64[:], in0=m128[0:64, :], in1=h64[:], op=AO.min
            )
            h32 = scr.tile([32, B], f32, tag="m128", name="h32")
            nc.vector.tensor_copy(out=h32[:], in_=m64[32:64, :])
            m32 = scr.tile([32, B], f32, tag="m128", name="m32")
            nc.vector.tensor_tensor(
                out=m32[:], in0=m64[0:32, :], in1=h32[:], op=AO.min
            )
            m32t = scr.tile([32, B], f32, tag="m128", name="m32t")
            nc.vector.transpose(out=m32t[:], in_=m32[:])
            # m32t[q, j*32 + r] = m32[r, j*32 + q]; reduce r -> min over partitions
            res32 = small.tile([32, 32], f32, name="res32", tag="res32")
            nc.vector.tensor_reduce(
                out=res32[:],
                in_=m32t[:].rearrange("p (j r) -> p j r", r=32),
                axis=mybir.AxisListType.X,
                op=AO.min,
            )
            if stage == 31:
                _dbg_out(res32[0:1, 0:1])
                raise _StageDone()
            # column c = j*32 + q of the original lives at res32[q, j]
            nc.sync.dma_start(
                bMin[:].rearrange("o (j q) -> (o q) j", q=32), res32[:]
            )
            nc.gpsimd.collective_compute(
                "AllGather", AO.bypass, replica_groups=RG,
                ins=[bMin[:]], outs=[bMinR[:]],
            )

            if stage <= 3:
                _dbg_out(res32[0:1, 0:1])
                raise _StageDone()

            # diag(c) local: row dots of Qa^T o Qb^T
            cd = small.tile([P, 2], f32, name="cd", tag="cd")
            for m in range(2):
                cscr = scr.tile([P, B], f32, tag="m128", name=f"cscr_{m}")
                nc.vector.tensor_tensor(
                    out=cscr[:], in0=q["a"][:, m, :], in1=q["b"][:, m, :], op=AO.mult
                )
                nc.vector.tensor_reduce(
                    out=cd[:, m : m + 1],
                    in_=cscr[:],
                    axis=mybir.AxisListType.X,
                    op=AO.add,
                )
            cdn = small.tile([P, 2], f32, name="cdn", tag="cdn")
            nc.vector.tensor_scalar(
                out=cdn[:], in0=cd[:], scalar1=1.0 / B, scalar2=None, op0=AO.mult
            )
            cm1 = small.tile([P, 2], f32, name="cm1", tag="cm1")
            nc.vector.tensor_scalar(
                out=cm1[:], in0=cdn[:], scalar1=1.0, scalar2=None, op0=AO.subtract
            )
            od2 = small.tile([P, 2], f32, name="od2", tag="od2")
            nc.vector.tensor_tensor(out=od2[:], in0=cm1[:], in1=cm1[:], op=AO.mult)
            dsq2 = small.tile([P, 2], f32, name="dsq2", tag="dsq2")
            nc.vector.tensor_tensor(out=dsq2[:], in0=cdn[:], in1=cdn[:], op=AO.mult)
            nc.vector.tensor_reduce(
                out=sums[:, 1:2], in_=od2[:], axis=mybir.AxisListType.X, op=AO.add
            )
            nc.vector.tensor_reduce(
                out=sums[:, 2:3], in_=dsq2[:], axis=mybir.AxisListType.X, op=AO.add
            )

            if stage <= 4:
                _dbg_out(q["b"][0:1, 0, 0:1])
                raise _StageDone()
            if stage <= 5:
                _dbg_out(sums[0:1, 0:1])
                raise _StageDone()

            # ------------- c band: Qa_loc @ QbT_full (fp8 DoubleRow) ----------
            # accumulate over the two gathered batch-half chunks
            qbF = protp.tile([P, 8, N_PROTO], fp8, name="qbF", tag="qbF")
            for h in range(2):
                for r in range(NCORES):
                    nc.sync.dma_start(
                        qbF[:, 4 * h : 4 * h + 4, r * NSH : (r + 1) * NSH],
                        bQtR[h][r * (B // 2) : (r + 1) * (B // 2), :].rearrange(
                            "(ko ki) n -> ki ko n", ki=P
                        ),
                    )
            cacc = small.tile([P, 8], f32, name="cacc", tag="cacc")
            for m in range(2):
                for nh in range(4):
                    pcs = psum.tile([P, 512], f32, tag="mm", name=f"c_{m}_{nh}")
                    for h in range(2):
                        for kg in range(2):
                            kk = 4 * h + 2 * kg
                            nc.tensor.matmul(
                                pcs[:],
                                qT["a"][:, kk : kk + 2, m * P : (m + 1) * P],
                                qbF[:, kk : kk + 2, nh * 512 : (nh + 1) * 512],
                                start=(h == 0 and kg == 0),
                                stop=(h == 1 and kg == 1),
                                perf_mode=DR,
                            )
                    csq = scr.tile([P, 512], f32, tag="dfs", name=f"csq_{m}_{nh}")
                    nc.scalar.square(out=csq[:], in_=pcs[:])
                    nc.vector.tensor_reduce(
                        out=cacc[:, m * 4 + nh : m * 4 + nh + 1],
                        in_=csq[:],
                        axis=mybir.AxisListType.X,
                        op=AO.add,
                    )
            nc.vector.tensor_reduce(
                out=sums[:, 4:5], in_=cacc[:], axis=mybir.AxisListType.X, op=AO.add
            )

            if stage <= 6:
                _dbg_out(sums[0:1, 0:1])
                raise _StageDone()

            # ------------- error_2: min over gathered per-core mins -----------
            # bMinR[r, j*32+q] -> gm32[q, r, j]; reduce r (cores), then sum
            gm32 = small.tile([32, NCORES, 32], f32, name="gm32", tag="gm32")
            nc.sync.dma_start(
                gm32[:], bMinR[:].rearrange("r (j q) -> q r j", q=32)
            )
            rsb = small.tile([32, 32], f32, name="rsb", tag="rsb")
            nc.vector.tensor_reduce(
                out=rsb[:],
                in_=gm32[:].rearrange("q r j -> q j r"),
                axis=mybir.AxisListType.X,
                op=AO.min,
            )
            nc.vector.tensor_reduce(
                out=sums[0:32, 5:6], in_=rsb[:], axis=mybir.AxisListType.X, op=AO.add
            )

            # ------------- pack + final AllGather + local reduce --------------
            fin = psa2.tile([1, 8], f32, tag="a2", name="fin")
            nc.tensor.matmul(
                fin[:],
                ones_f32[:],
                sums[:],
                start=True,
                stop=True,
            )
            pack = small.tile([1, 8], f32, name="pack", tag="pack")
            nc.vector.memset(pack[:], 0.0)
            for col, (part, coef) in enumerate(
                [
                    (fin[0:1, 0:1], 1.0 / N_PROTO),         # error_1
                    (fin[0:1, 5:6], 1.0 / (B * NCORES)),    # error_2 (replicated)
                    (fin[0:1, 3:4], 1.0 / B),               # vae
                    (fin[0:1, 1:2], 1.0),                   # on_diag
                    (fin[0:1, 4:5], LAMBD / (B * B)),       # lambd * sum c^2
                    (fin[0:1, 2:3], -LAMBD),                # -lambd * sum diag^2
                ]
            ):
                nc.vector.tensor_scalar(
                    out=pack[:, col : col + 1],
                    in0=part,
                    scalar1=coef,
                    scalar2=None,
                    op0=AO.mult,
                )
            nc.sync.dma_start(bPack[:], pack[:])
            nc.gpsimd.collective_compute(
                "AllGather",
                AO.bypass,
                replica_groups=RG,
                ins=[bPack[:]],
                outs=[bPackR[:]],
            )
            pr = small.tile([8, 8], f32, name="pr", tag="pr")
            nc.sync.dma_start(pr[:], bPackR[:])
            prr = psa2.tile([1, 8], f32, tag="a2", name="prr")
            nc.tensor.matmul(
                prr[:],
                ones_f32[0:8, :],
                pr[:],
                start=True,
                stop=True,
            )
            res = small.tile([1, 1], f32, name="res", tag="res")
            nc.vector.tensor_reduce(
                out=res[:], in_=prr[:], axis=mybir.AxisListType.X, op=AO.add
            )
            nc.sync.dma_start(out[:], res[:])

      except _StageDone:
          pass
    return


def _get_program(stage=99):
    key = ("nc", stage)
    if key not in _PROG_CACHE:
        _PROG_CACHE[key] = _build_program(stage)
    return _PROG_CACHE[key]


def _make_in_maps(ds_one, ds_two, W_enc, W_dec, prototypes):
    p2 = (prototypes * prototypes).sum(axis=1)
    in_maps = []
    for c in range(NCORES):
        dsl = slice(c * DSH, (c + 1) * DSH)
        nsl = slice(c * NSH, (c + 1) * NSH)
        prp = np.zeros((KAUG, NSH), np.float32)
        prp[0:P_DIM, :] = -2.0 * prototypes[nsl, :].T
        prp[P_DIM, :] = p2[nsl] / WDS        # pairs zaug's 16-row
        prp[P_DIM + 32, :] = WDS             # pairs zaug's a2/16 row
        in_maps.append(
            {
                "dsa": np.ascontiguousarray(ds_one[:, dsl].T).astype(BF16),
                "dsb": np.ascontiguousarray(ds_two[:, dsl].T).astype(BF16),
                "wenc": np.ascontiguousarray(W_enc[dsl, :]).astype(BF16),
                "wdec": np.ascontiguousarray(W_dec[:, dsl] * WDS).astype(F8),
                "prp": prp.astype(F8),
            }
        )
    return in_maps


def kernel(ds_one, ds_two, W_enc, W_dec, prototypes, _trace=False, _tmpdir=None):
    import os
    from concourse import bass_utils

    ds_one = np.asarray(ds_one, np.float32)
    ds_two = np.asarray(ds_two, np.float32)
    W_enc = np.asarray(W_enc, np.float32)
    W_dec = np.asarray(W_dec, np.float32)
    prototypes = np.asarray(prototypes, np.float32)

    nc = _get_program(int(os.environ.get("KSTAGE", "99")))
    in_maps = _make_in_maps(ds_one, ds_two, W_enc, W_dec, prototypes)
    res = bass_utils.run_bass_kernel_spmd(
        nc,
        in_maps,
        core_ids=list(range(NCORES)),
        trace=_trace,
        tmpdir=_tmpdir,
    )
    val = np.asarray(res.results[0]["out"], np.float32)
    if _trace:
        kernel.last_exec_time_ns = res.exec_time_ns
        kernel.last_profile = res.profile_json
    return val.reshape(())


# revision 24
# speedup vs baseline: 1.1672x; 1.1633x over previous
"""Trainium2 Bass kernel for nn_PrototypeBarlow (vq_codebook).

Sharding (8 cores):
  - Encode (bf16): shard D_IMG (contraction); per-core partial z^T [P_DIM, B]
    per dataset, AllReduce(add) bf16 per dataset; a's AR overlaps b's encode,
    and both ARs hide behind tensor work. Input DMAs chunked per k-tile.
  - Prototypes (fp8 DoubleRow): shard N_PROTO; augmented matmul on
    zaug = [z; 16; z^2/16; 0-pad] vs prp = [-2 proto^T; p2/16; 16; 0-pad]
    (scales keep every fp8 operand under the 240 max-normal).
  - Per-dataset chain zaug -> pt -> stats -> norm -> transpose -> decode
    pipelines against the other dataset's AllReduce.
  - Barlow: transpose Q (TensorE identity matmul) -> [B, 256] fp8; AllGather
    Qb^T in two 128KB chunks; c-band matmul (fp8 DoubleRow) accumulates per
    chunk; square+sum. diag(c) local row dots in bf16.
  - VAE (fp8 DoubleRow decode, W_dec pre-scaled x16): fused
    (dec/16 - ds) via scalar_tensor_tensor, square on ACT engine, reduce on
    DVE.
  - error_1: free-axis min + local sum. error_2: partition-tree min ->
    [1,B], AllGather + local min tree (cheaper than AllReduce-min).
  - Final: pre-scaled scalar partials [1,8], AllGather -> [8,8], ones-matmul.
"""

import numpy as np
import ml_dtypes

BF16 = ml_dtypes.bfloat16
F8 = ml_dtypes.float8_e4m3

B = 1024
D_IMG = 12288
P_DIM = 512
N_PROTO = 2048
NCORES = 8
DSH = D_IMG // NCORES    # 1536
NSH = N_PROTO // NCORES  # 256
KAUG = 768               # 512 + aug rows, padded to 6*128 (even for DoubleRow)
LAMBD = 0.005
EPS = 1e-5
WDS = 16.0               # host pre-scale on W_dec / aug rows for fp8 range

_PROG_CACHE = {}


def _build_program(stage=99):
    import concourse.bacc as bacc
    import concourse.tile as tile
    from concourse import mybir

    class _StageDone(Exception):
        pass

    nc = bacc.Bacc("TRN2", target_bir_lowering=False, num_devices=NCORES)

    try:
        _run_build(nc, tile, mybir, stage, _StageDone)
    except _StageDone:
        pass
    nc.finalize()
    return nc


def _run_build(nc, tile, mybir, stage, _StageDone):
    from contextlib import ExitStack
    from concourse.masks import make_identity

    dt = mybir.dt
    f32 = dt.float32
    bft = dt.bfloat16
    fp8 = dt.float8e4
    AO = mybir.AluOpType
    DR = mybir.MatmulPerfMode.DoubleRow
    P = 128
    RG = [list(range(NCORES))]
    dsa = nc.dram_tensor("dsa", [DSH, B], bft, kind="ExternalInput")
    dsb = nc.dram_tensor("dsb", [DSH, B], bft, kind="ExternalInput")
    wenc = nc.dram_tensor("wenc", [DSH, P_DIM], bft, kind="ExternalInput")
    wdec = nc.dram_tensor("wdec", [P_DIM, DSH], fp8, kind="ExternalInput")
    prp = nc.dram_tensor("prp", [KAUG, NSH], fp8, kind="ExternalInput")
    out = nc.dram_tensor("out", [1, 1], f32, kind="ExternalOutput")

    with tile.TileContext(nc) as tc, ExitStack() as ctx:
      try:
            dram = ctx.enter_context(tc.tile_pool(name="dram", bufs=1, space="DRAM"))
            bZ = {}
            bZR = {}
            for s in "ab":
                bZ[s] = dram.tile([P_DIM, B], fp8, name=f"bZ{s}", tag=f"bZ{s}")
                bZR[s] = dram.tile(
                    [P_DIM, B], fp8, addr_space="Shared", name=f"bZR{s}", tag=f"bZR{s}"
                )
            # Qb^T gathered in two batch-half chunks
            bQt = {}
            bQtR = {}
            for h in range(2):
                bQt[h] = dram.tile([B // 2, NSH], fp8, name=f"bQt{h}", tag=f"bQt{h}")
                bQtR[h] = dram.tile(
                    [B * NCORES // 2, NSH], fp8, addr_space="Shared",
                    name=f"bQtR{h}", tag=f"bQtR{h}",
                )
            bMin = dram.tile([1, B], f32, name="bMin", tag="bMin")
            bMinR = dram.tile(
                [NCORES, B], f32, addr_space="Shared", name="bMinR", tag="bMinR"
            )
            bPack = dram.tile([1, 8], f32, name="bPack", tag="bPack")
            bPackR = dram.tile(
                [NCORES, 8], f32, addr_space="Shared", name="bPackR", tag="bPackR"
            )

            const = ctx.enter_context(tc.tile_pool(name="const", bufs=1))
            wenc_sb = const.tile([P, 12, P_DIM], bft, name="wenc_sb", tag="wbig")
            wenc_r = wenc[:].rearrange("(ko ki) n -> ki ko n", ki=P)
            for k in range(12):
                nc.sync.dma_start(wenc_sb[:, k, :], wenc_r[:, k, :])
            prp_sb = const.tile([P, 6, NSH], fp8, name="prp_sb", tag="prp_sb")
            nc.sync.dma_start(prp_sb[:], prp[:].rearrange("(ko ki) n -> ki ko n", ki=P))
            ones_col = const.tile([P, 1], bft, name="ones_col", tag="ones_col")
            nc.vector.memset(ones_col[:], 1.0)
            ones_f32 = const.tile([P, 1], f32, name="ones_f32", tag="ones_f32")
            nc.vector.memset(ones_f32[:], 1.0)
            ident = const.tile([P, P], bft, name="ident", tag="ident")
            make_identity(nc, ident[:])
            identn = const.tile([P, P], bft, name="identn", tag="identn")
            nc.vector.tensor_scalar(
                out=identn[:], in0=ident[:], scalar1=-WDS, scalar2=None, op0=AO.mult
            )

            dsp = ctx.enter_context(tc.tile_pool(name="dsp", bufs=1))
            ds_sb = {}
            for s in "ab":
                ds_sb[s] = dsp.tile([P, 12, B], bft, name=f"ds{s}_sb", tag=f"ds{s}_sb")
            dsa_r = dsa[:].rearrange("(ko ki) b -> ki ko b", ki=P)
            for k in range(12):
                nc.sync.dma_start(ds_sb["a"][:, k, :], dsa_r[:, k, :])

            psum = ctx.enter_context(tc.tile_pool(name="psum", bufs=6, space="PSUM"))
            psa2 = ctx.enter_context(tc.tile_pool(name="psa2", bufs=1, space="PSUM"))
            zp = ctx.enter_context(tc.tile_pool(name="zp", bufs=1))
            protp = ctx.enter_context(tc.tile_pool(name="protp", bufs=1))
            scr = ctx.enter_context(tc.tile_pool(name="scr", bufs=3))
            small = ctx.enter_context(tc.tile_pool(name="small", bufs=1))
            evp = ctx.enter_context(tc.tile_pool(name="evp", bufs=4))

            # per-partition partial sums gathered as columns; reduced once at the end
            sums = small.tile([P, 8], f32, name="sums", tag="sums")
            nc.vector.memset(sums[:], 0.0)
            vacc = small.tile([P, 48], f32, name="vacc", tag="vacc")

            # ---------------- encode:  zT_part = wenc^T @ dsT (bf16) ----------
            for s in "ab":
                src = ds_sb[s]
                bZt = bZ[s][:].rearrange("(ko ki) b -> ki ko b", ki=P)
                for mg in range(2):
                    pts = {}
                    for mi in range(2):
                        for n in range(2):
                            pts[(mi, n)] = psum.tile(
                                [P, 512], f32, tag="mm", name=f"enc_{s}_{mg}_{mi}_{n}"
                            )
                    for k in range(12):
                        for mi in range(2):
                            m = mg * 2 + mi
                            for n in range(2):
                                nc.tensor.matmul(
                                    pts[(mi, n)][:],
                                    wenc_sb[:, k, m * P : (m + 1) * P],
                                    src[:, k, n * 512 : (n + 1) * 512],
                                    start=(k == 0),
                                    stop=(k == 11),
                                )
                    for mi in range(2):
                        m = mg * 2 + mi
                        for n in range(2):
                            ev = evp.tile([P, 512], fp8, tag="ev", name=f"ev_{s}_{m}_{n}")
                            nc.any.tensor_copy(out=ev[:], in_=pts[(mi, n)][:])
                            nc.sync.dma_start(bZt[:, m, n * 512 : (n + 1) * 512], ev[:])
                nc.gpsimd.collective_compute(
                    "AllReduce",
                    AO.add,
                    replica_groups=RG,
                    ins=[bZ[s][:]],
                    outs=[bZR[s][:]],
                )
                if s == "a":
                    dsb_r = dsb[:].rearrange("(ko ki) b -> ki ko b", ki=P)
                    for k in range(12):
                        nc.sync.dma_start(ds_sb["b"][:, k, :], dsb_r[:, k, :])

            # wdec reuses wenc's SBUF slot once the encode matmuls are done
            wdec_sb = const.tile([P, 4, DSH], fp8, name="wdec_sb", tag="wbig")
            nc.sync.dma_start(wdec_sb[:], wdec[:].rearrange("(ko ki) n -> ki ko n", ki=P))

            def _dbg_out(ap):
                dbg = small.tile([1, 1], f32, name="dbg", tag="dbg")
                nc.vector.tensor_copy(out=dbg[:], in_=ap)
                nc.sync.dma_start(out[:], dbg[:])

            # ---- per-dataset: zaug -> pt -> stats -> norm -> qT -> decode ----
            zaug = {}
            pt = {}
            q = {}
            qT = {}
            for si, s in enumerate("ab"):
                # zaug (fp8) = [z; 16; a2/16; pad], z DMA'd straight from
                # the fp8-reduced buffer
                za = zp.tile([P, 6, B], fp8, name=f"zaug_{s}", tag=f"zaug_{s}")
                zaug[s] = za
                nc.sync.dma_start(
                    za[:, 0:4, :], bZR[s][:].rearrange("(ko ki) b -> ki ko b", ki=P)
                )
                nc.vector.memset(za[:, 4:6, :], 0.0)
                nc.vector.memset(za[0:1, 4, :], WDS)
                zsq = scr.tile([P, 4, B], bft, tag="zsq", name=f"zsq_{s}", bufs=2)
                nc.vector.tensor_tensor(
                    out=zsq[:], in0=za[:, 0:4, :], in1=za[:, 0:4, :], op=AO.mult
                )
                pa2 = psa2.tile([1, 2, 512], f32, tag="a2", name=f"a2_{s}")
                for k in range(4):
                    for n in range(2):
                        nc.tensor.matmul(
                            pa2[:, n, :],
                            ones_col[:],
                            zsq[:, k, n * 512 : (n + 1) * 512],
                            start=(k == 0),
                            stop=(k == 3),
                        )
                # a2/16 lands on partition 32 of aug chunk 4 (pairs prp row 544=16)
                nc.vector.tensor_scalar(
                    out=za[32:33, 4, :],
                    in0=pa2[0:1, :, :],
                    scalar1=1.0 / WDS,
                    scalar2=None,
                    op0=AO.mult,
                )

                # protT = prp^T @ zaug   [256, B] f32   (fp8 DoubleRow, K=768)
                ptile = protp.tile([P, 2, B], f32, name=f"pt_{s}", tag=f"pt_{s}")
                pt[s] = ptile
                for m in range(2):
                    pps = {}
                    for n in range(2):
                        pps[n] = psum.tile([P, 512], f32, tag="mm", name=f"pr_{s}_{m}_{n}")
                    for kg in range(3):
                        for n in range(2):
                            nc.tensor.matmul(
                                pps[n][:],
                                prp_sb[:, 2 * kg : 2 * kg + 2, m * P : (m + 1) * P],
                                zaug[s][:, 2 * kg : 2 * kg + 2, n * 512 : (n + 1) * 512],
                                start=(kg == 0),
                                stop=(kg == 2),
                                perf_mode=DR,
                            )
                    for n in range(2):
                        nc.any.tensor_copy(
                            out=ptile[:, m, n * 512 : (n + 1) * 512], in_=pps[n][:]
                        )

                # barlow stats + normalize
                qt = protp.tile([P, 2, B], bft, name=f"q_{s}", tag=f"q_{s}")
                q[s] = qt
                for m in range(2):
                    st6 = small.tile(
                        [P, 2, 6], f32, tag="st6", name=f"st6_{s}_{m}", bufs=2
                    )
                    for c in range(2):
                        nc.vector.bn_stats(
                            out=st6[:, c, :], in_=pt[s][:, m, c * 512 : (c + 1) * 512]
                        )
                    mv = small.tile([P, 2], f32, tag="mv", name=f"mv_{s}_{m}", bufs=2)
                    nc.vector.bn_aggr(out=mv[:], in_=st6[:])
                    sd = small.tile([P, 1], f32, tag="sd", name=f"sd_{s}_{m}", bufs=2)
                    nc.scalar.sqrt(out=sd[:], in_=mv[:, 1:2])
                    sde = small.tile([P, 1], f32, tag="sde", name=f"sde_{s}_{m}", bufs=2)
                    nc.vector.tensor_scalar(
                        out=sde[:], in0=sd[:], scalar1=EPS, scalar2=None, op0=AO.add
                    )
                    rstd = small.tile([P, 1], f32, tag="rstd", name=f"rstd_{s}_{m}", bufs=2)
                    nc.vector.reciprocal(out=rstd[:], in_=sde[:])
                    nc.vector.tensor_scalar(
                        out=qt[:, m, :],
                        in0=pt[s][:, m, :],
                        scalar1=mv[:, 0:1],
                        scalar2=rstd[:],
                        op0=AO.subtract,
                        op1=AO.mult,
                    )

                # transpose q -> qT [B-part, 256] fp8
                qT[s] = protp.tile([P, 8, NSH], fp8, name=f"qT_{s}", tag=f"qT_{s}")
                for m in range(2):
                    for g in range(2):
                        ptr = psum.tile(
                            [P, 4, P], bft, tag="mm", name=f"tp_{s}_{m}_{g}"
                        )
                        for kk in range(4):
                            kb = g * 4 + kk
                            nc.tensor.transpose(
                                ptr[:, kk, :],
                                q[s][:, m, kb * P : (kb + 1) * P],
                                ident[:],
                            )
                        nc.any.tensor_copy(
                            out=qT[s][:, g * 4 : (g + 1) * 4, m * P : (m + 1) * P],
                            in_=ptr[:],
                        )

                if s == "b":
                    # ship Qb^T for the AllGather in two batch-half chunks
                    for h in range(2):
                        nc.sync.dma_start(
                            bQt[h][:].rearrange("(ko ki) n -> ki ko n", ki=P),
                            qT["b"][:, 4 * h : 4 * h + 4, :],
                        )
                        nc.gpsimd.collective_compute(
                            "AllGather",
                            AO.bypass,
                            replica_groups=RG,
                            ins=[bQt[h][:]],
                            outs=[bQtR[h][:]],
                        )

                # VAE decode for this dataset (fp8 DoubleRow + folded subtract);
                # dec_a fills the z_b AllReduce bubble, dec_b hides the gather
                for m in range(12):
                    pps = {}
                    for n in range(2):
                        pps[n] = psum.tile([P, 512], f32, tag="mm", name=f"d_{s}_{m}_{n}")
                    for kg in range(2):
                        for n in range(2):
                            nc.tensor.matmul(
                                pps[n][:],
                                wdec_sb[:, 2 * kg : 2 * kg + 2, m * P : (m + 1) * P],
                                zaug[s][:, 2 * kg : 2 * kg + 2, n * 512 : (n + 1) * 512],
                                start=(kg == 0),
                                stop=False,
                                perf_mode=DR,
                            )
                    # PSUM -= 16*ds  ->  PSUM holds 16*(dec - ds)
                    for n in range(2):
                        nc.tensor.matmul(
                            pps[n][:],
                            identn[:],
                            ds_sb[s][:, m, n * 512 : (n + 1) * 512],
                            start=False,
                            stop=True,
                        )
                    # sum((psum/16)^2) per partition, all on the ACT engine
                    for n in range(2):
                        dfs = scr.tile([P, 512], bft, tag="dfs", name=f"dfs_{s}_{m}_{n}")
                        col = si * 24 + m * 2 + n
                        nc.scalar.activation(
                            out=dfs[:],
                            in_=pps[n][:],
                            func=mybir.ActivationFunctionType.Square,
                            bias=0.0,
                            scale=1.0 / WDS,
                            accum_out=vacc[:, col : col + 1],
                        )


            # ---------------- mins on s = prot_a + prot_b ---------------------
            sT = scr.tile([P, 2, B], f32, tag="zsq", name="sT", bufs=2)
            minb = small.tile([P, 2], f32, name="minb", tag="minb")
            for m in range(2):
                nc.vector.tensor_tensor(
                    out=sT[:, m, :],
                    in0=pt["a"][:, m, :],
                    in1=pt["b"][:, m, :],
                    op=AO.add,
                )
                nc.vector.tensor_reduce(
                    out=minb[:, m : m + 1],
                    in_=sT[:, m, :],
                    axis=mybir.AxisListType.X,
                    op=AO.min,
                )
            # error_1 partial: sum over local prototypes of min over batch
            nc.vector.tensor_reduce(
                out=sums[:, 0:1], in_=minb[:], axis=mybir.AxisListType.X, op=AO.add
            )
            if stage == 30:
                _dbg_out(minb[0:1, 0:1])
                raise _StageDone()
            # error_2: min over local protos across partitions -> [1, B]:
            # fold 128->32, then 32x32 stream-transpose + free-axis min
            m128 = scr.tile([P, B], f32, tag="m128", name="m128")
            nc.vector.tensor_tensor(
                out=m128[:], in0=sT[:, 0, :], in1=sT[:, 1, :], op=AO.min
            )
            h64 = scr.tile([64, B], f32, tag="m128", name="h64")
            nc.vector.tensor_copy(out=h64[:], in_=m128[64:128, :])
            m64 = scr.tile([64, B], f32, tag="m128", name="m64")
            nc.vector.tensor_tensor(
                out=m64[:], in0=m128[0:64, :], in1=h64[:], op=AO.min
            )
            h32 = scr.tile([32, B], f32, tag="m128", name="h32")
            nc.vector.tensor_copy(out=h32[:], in_=m64[32:64, :])
            m32 = scr.tile([32, B], f32, tag="m128", name="m32")
            nc.vector.tensor_tensor(
                out=m32[:], in0=m64[0:32, :], in1=h32[:], op=AO.min
            )
            m32t = scr.tile([32, B], f32, tag="m128", name="m32t")
            nc.vector.transpose(out=m32t[:], in_=m32[:])
            # m32t[q, j*32 + r] = m32[r, j*32 + q]; reduce r -> min over partitions
            res32 = small.tile([32, 32], f32, name="res32", tag="res32")
            nc.vector.tensor_reduce(
                out=res32[:],
                in_=m32t[:].rearrange("p (j r) -> p j r", r=32),
                axis=mybir.AxisListType.X,
                op=AO.min,
            )
            if stage == 31:
                _dbg_out(res32[0:1, 0:1])
                raise _StageDone()
            # column c = j*32 + q of the original lives at res32[q, j]
            nc.sync.dma_start(
                bMin[:].rearrange("o (j q) -> (o q) j", q=32), res32[:]
            )
            nc.gpsimd.collective_compute(
                "AllGather", AO.bypass, replica_groups=RG,
                ins=[bMin[:]], outs=[bMinR[:]],
            )

            if stage <= 3:
                _dbg_out(res32[0:1, 0:1])
                raise _StageDone()

            # diag(c) local: row dots of Qa^T o Qb^T
            cd = small.tile([P, 2], f32, name="cd", tag="cd")
            for m in range(2):
                cscr = scr.tile([P, B], f32, tag="m128", name=f"cscr_{m}")
                nc.vector.tensor_tensor(
                    out=cscr[:], in0=q["a"][:, m, :], in1=q["b"][:, m, :], op=AO.mult
                )
                nc.vector.tensor_reduce(
                    out=cd[:, m : m + 1],
                    in_=cscr[:],
                    axis=mybir.AxisListType.X,
                    op=AO.add,
                )
            cdn = small.tile([P, 2], f32, name="cdn", tag="cdn")
            nc.vector.tensor_scalar(
                out=cdn[:], in0=cd[:], scalar1=1.0 / B, scalar2=None, op0=AO.mult
            )
            cm1 = small.tile([P, 2], f32, name="cm1", tag="cm1")
            nc.vector.tensor_scalar(
                out=cm1[:], in0=cdn[:], scalar1=1.0, scalar2=None, op0=AO.subtract
            )
            od2 = small.tile([P, 2], f32, name="od2", tag="od2")
            nc.vector.tensor_tensor(out=od2[:], in0=cm1[:], in1=cm1[:], op=AO.mult)
            dsq2 = small.tile([P, 2], f32, name="dsq2", tag="dsq2")
            nc.vector.tensor_tensor(out=dsq2[:], in0=cdn[:], in1=cdn[:], op=AO.mult)
            nc.vector.tensor_reduce(
                out=sums[:, 1:2], in_=od2[:], axis=mybir.AxisListType.X, op=AO.add
            )
            nc.vector.tensor_reduce(
                out=sums[:, 2:3], in_=dsq2[:], axis=mybir.AxisListType.X, op=AO.add
            )

            nc.vector.tensor_reduce(
                out=sums[:, 3:4], in_=vacc[:], axis=mybir.AxisListType.X, op=AO.add
            )

            if stage <= 1:
                _dbg_out(zaug["b"][0:1, 0, 0:1])
                raise _StageDone()
            if stage <= 2:
                _dbg_out(pt["b"][0:1, 0, 0:1])
                raise _StageDone()

            if stage <= 4:
                _dbg_out(q["b"][0:1, 0, 0:1])
                raise _StageDone()
            if stage <= 5:
                _dbg_out(sums[0:1, 0:1])
                raise _StageDone()

            # ------------- c band: Qa_loc @ QbT_full (fp8 DoubleRow) ----------
            # accumulate over the two gathered batch-half chunks
            qbF = protp.tile([P, 8, N_PROTO], fp8, name="qbF", tag="qbF")
            for h in range(2):
                for r in range(NCORES):
                    nc.sync.dma_start(
                        qbF[:, 4 * h : 4 * h + 4, r * NSH : (r + 1) * NSH],
                        bQtR[h][r * (B // 2) : (r + 1) * (B // 2), :].rearrange(
                            "(ko ki) n -> ki ko n", ki=P
                        ),
                    )
            cacc = small.tile([P, 8], f32, name="cacc", tag="cacc")
            for m in range(2):
                for nh in range(4):
                    pcs = psum.tile([P, 512], f32, tag="mm", name=f"c_{m}_{nh}")
                    for h in range(2):
                        for kg in range(2):
                            kk = 4 * h + 2 * kg
                            nc.tensor.matmul(
                                pcs[:],
                                qT["a"][:, kk : kk + 2, m * P : (m + 1) * P],
                                qbF[:, kk : kk + 2, nh * 512 : (nh + 1) * 512],
                                start=(h == 0 and kg == 0),
                                stop=(h == 1 and kg == 1),
                                perf_mode=DR,
                            )
                    csq = scr.tile([P, 512], bft, tag="dfs", name=f"csq_{m}_{nh}")
                    nc.scalar.activation(
                        out=csq[:],
                        in_=pcs[:],
                        func=mybir.ActivationFunctionType.Square,
                        bias=0.0,
                        scale=1.0,
                        accum_out=cacc[:, m * 4 + nh : m * 4 + nh + 1],
                    )
            nc.vector.tensor_reduce(
                out=sums[:, 4:5], in_=cacc[:], axis=mybir.AxisListType.X, op=AO.add
            )

            if stage <= 6:
                _dbg_out(sums[0:1, 0:1])
                raise _StageDone()

            # ------------- error_2: min over gathered per-core mins -----------
            # bMinR[r, j*32+q] -> gm32[q, r, j]; reduce r (cores), then sum
            gm32 = small.tile([32, NCORES, 32], f32, name="gm32", tag="gm32")
            nc.sync.dma_start(
                gm32[:], bMinR[:].rearrange("r (j q) -> q r j", q=32)
            )
            rsb = small.tile([32, 32], f32, name="rsb", tag="rsb")
            nc.vector.tensor_reduce(
                out=rsb[:],
                in_=gm32[:].rearrange("q r j -> q j r"),
                axis=mybir.AxisListType.X,
                op=AO.min,
            )
            r32c = small.tile([32, 1], f32, name="r32c", tag="r32c")
            nc.vector.tensor_reduce(
                out=r32c[:], in_=rsb[:], axis=mybir.AxisListType.X, op=AO.add
            )
            e2v = psa2.tile([1, 1], f32, tag="a2", name="e2v")
            nc.tensor.matmul(
                e2v[:],
                ones_f32[0:32, :],
                r32c[:],
                start=True,
                stop=True,
            )

            # ------------- pack + final AllGather + local reduce --------------
            fin = psa2.tile([1, 8], f32, tag="a2", name="fin")
            nc.tensor.matmul(
                fin[:],
                ones_f32[:],
                sums[:],
                start=True,
                stop=True,
            )
            pack = small.tile([1, 8], f32, name="pack", tag="pack")
            nc.vector.memset(pack[:], 0.0)
            for col, (part, coef) in enumerate(
                [
                    (e2v[:], 1.0 / (B * NCORES)),           # error_2 (replicated)
                    (fin[0:1, 0:1], 1.0 / N_PROTO),         # error_1
                    (fin[0:1, 3:4], 1.0 / B),               # vae
                    (fin[0:1, 1:2], 1.0),                   # on_diag
                    (fin[0:1, 4:5], LAMBD / (B * B)),       # lambd * sum c^2
                    (fin[0:1, 2:3], -LAMBD),                # -lambd * sum diag^2
                ]
            ):
                nc.vector.tensor_scalar(
                    out=pack[:, col : col + 1],
                    in0=part,
                    scalar1=coef,
                    scalar2=None,
                    op0=AO.mult,
                )
            nc.sync.dma_start(bPack[:], pack[:])
            nc.gpsimd.collective_compute(
                "AllGather",
                AO.bypass,
                replica_groups=RG,
                ins=[bPack[:]],
                outs=[bPackR[:]],
            )
            pr = small.tile([8, 8], f32, name="pr", tag="pr")
            nc.sync.dma_start(pr[:], bPackR[:])
            prr = psa2.tile([1, 8], f32, tag="a2", name="prr")
            nc.tensor.matmul(
                prr[:],
                ones_f32[0:8, :],
                pr[:],
                start=True,
                stop=True,
            )
            res = small.tile([1, 1], f32, name="res", tag="res")
            nc.vector.tensor_reduce(
                out=res[:], in_=prr[:], axis=mybir.AxisListType.X, op=AO.add
            )
            nc.sync.dma_start(out[:], res[:])

      except _StageDone:
          pass
    return


def _get_program(stage=99):
    key = ("nc", stage)
    if key not in _PROG_CACHE:
        _PROG_CACHE[key] = _build_program(stage)
    return _PROG_CACHE[key]


def _make_in_maps(ds_one, ds_two, W_enc, W_dec, prototypes):
    p2 = (prototypes * prototypes).sum(axis=1)
    in_maps = []
    for c in range(NCORES):
        dsl = slice(c * DSH, (c + 1) * DSH)
        nsl = slice(c * NSH, (c + 1) * NSH)
        prp = np.zeros((KAUG, NSH), np.float32)
        prp[0:P_DIM, :] = -2.0 * prototypes[nsl, :].T
        prp[P_DIM, :] = p2[nsl] / WDS        # pairs zaug's 16-row
        prp[P_DIM + 32, :] = WDS             # pairs zaug's a2/16 row
        in_maps.append(
            {
                "dsa": np.ascontiguousarray(ds_one[:, dsl].T).astype(BF16),
                "dsb": np.ascontiguousarray(ds_two[:, dsl].T).astype(BF16),
                "wenc": np.ascontiguousarray(W_enc[dsl, :]).astype(BF16),
                "wdec": np.ascontiguousarray(W_dec[:, dsl] * WDS).astype(F8),
                "prp": prp.astype(F8),
            }
        )
    return in_maps


def kernel(ds_one, ds_two, W_enc, W_dec, prototypes, _trace=False, _tmpdir=None):
    import os
    from concourse import bass_utils

    ds_one = np.asarray(ds_one, np.float32)
    ds_two = np.asarray(ds_two, np.float32)
    W_enc = np.asarray(W_enc, np.float32)
    W_dec = np.asarray(W_dec, np.float32)
    prototypes = np.asarray(prototypes, np.float32)

    nc = _get_program(int(os.environ.get("KSTAGE", "99")))
    in_maps = _make_in_maps(ds_one, ds_two, W_enc, W_dec, prototypes)
    res = bass_utils.run_bass_kernel_spmd(
        nc,
        in_maps,
        core_ids=list(range(NCORES)),
        trace=_trace,
        tmpdir=_tmpdir,
    )
    val = np.asarray(res.results[0]["out"], np.float32)
    if _trace:
        kernel.last_exec_time_ns = res.exec_time_ns
        kernel.last_profile = res.profile_json
    return val.reshape(())
